# revision 4
# baseline (speedup 1.0000x reference)
"""Trainium2 Bass kernel for the KeypointLoss problem.

Full inputs:
  combined_preds [16, 4, 22, 128, 128] f32
  heatmaps       [16, 11, 128, 128]    f32
  labels         [16, 11, 11]          f32
Outputs (matching the reference):
  heat_loss  [16, 4] f32
  label_loss [16, 4] f32

Sharding: pure data parallel over the batch dim — core i handles batches
[2i, 2i+2). Each core computes its [2, 4] slices of both losses.

Per-core algorithm (B_L=2 local batches, S=4 stacks, K=11 keypoints,
H=W=128; G = B_L*S = 8 groups, PL = G*K = 88 planes):

  heat_loss[b,s] = sum_khw (hm - ht)^2
                 = sum hm^2 - 2*sum hm*ht + sum ht^2      (expansion)
    - sum hm^2, sum ht^2: ScalarE Square with accum_out (per-partition sums)
    - cross term: TensorE matmuls ht_k^T @ hm_k accumulated over k in PSUM,
      diagonal extracted via DVE scalar_tensor_tensor with identity + accum
    - partition sums via ones^T matmul

  label_loss needs per-plane argmax of hm over (h, w):
    - row max R[h, plane] via DVE tensor_reduce(max) over w
    - transpose R on PE -> [plane, h]; M = max_h; x = sum h*(R==M)
    - gather row x of each plane from DRAM via indirect DMA
    - y = sum w*(row==M)
    then the small per-plane class/xy/conf losses, masked by validity,
    reduced over k via a PE transpose + DVE segment reduce.

Only the hm half of combined_preds (plus 9 elements per plane of the lb
half) is ever read from HBM: ~6.5 MB per core.
"""

import sys

for _p in ("/opt/trn_rl_repo", "/root/.axon_site/_ro/trn_rl_repo"):
    if _p not in sys.path:
        sys.path.append(_p)

from contextlib import ExitStack

import numpy as np

# Problem constants (hardcoded per the task contract).
B, S, K, H, W = 16, 4, 11, 128, 128
NCORES = 8
BL = B // NCORES          # local batch per core = 2
G = BL * S                # groups per core = 8
PL = G * K                # planes per core = 88
KW = K * W                # free size of one group tile = 1408
C2 = 2 * K                # channel count of combined_preds = 22

_CACHE = {}


def _build_module():
    import concourse.bass as bass
    import concourse.tile as tile
    from concourse import bacc, mybir

    f32 = mybir.dt.float32
    Alu = mybir.AluOpType
    Act = mybir.ActivationFunctionType
    Ax = mybir.AxisListType

    nc = bacc.Bacc("TRN2", debug=False, enable_asserts=False, num_devices=NCORES)

    cp = nc.dram_tensor("cp", [BL, S, C2, H, W], f32, kind="ExternalInput").ap()
    hmr = nc.dram_tensor("hmr", [BL, K, H, W], f32, kind="ExternalInput").ap()
    lbl = nc.dram_tensor("lbl", [BL, K, 11], f32, kind="ExternalInput").ap()
    out_heat = nc.dram_tensor("out_heat", [1, G], f32, kind="ExternalOutput").ap()
    out_label = nc.dram_tensor("out_label", [1, G], f32, kind="ExternalOutput").ap()

    # Inline constants (shipped in the NEFF).
    ident = nc.inline_tensor(np.eye(128, dtype=np.float32), "identc").ap()
    iota = nc.inline_tensor(
        np.tile(np.arange(128, dtype=np.float32), (128, 1)), "iotac"
    ).ap()
    ones = nc.inline_tensor(np.ones((128, 1), np.float32), "onesc").ap()
    # DRAM row index (in units of W-element rows) of (plane, h=0) within cp
    # viewed as [(BL*S*C2*H), W].
    rb = np.zeros((PL, 1), np.float32)
    for g in range(G):
        b, s = divmod(g, S)
        for k in range(K):
            rb[g * K + k, 0] = ((b * S + s) * C2 + k) * H
    rbase = nc.inline_tensor(rb, "rbasec").ap()

    with tile.TileContext(nc) as tc, ExitStack() as ctx:
        sb = ctx.enter_context(tc.tile_pool(name="sb", bufs=1))
        scr = ctx.enter_context(tc.tile_pool(name="scr", bufs=2))
        ps = ctx.enter_context(tc.tile_pool(name="ps", bufs=1, space="PSUM"))

        # ---- constant / small input loads ----
        id_t = sb.tile([128, 128], f32, name="id_t")
        nc.sync.dma_start(id_t[:], ident)
        io_t = sb.tile([128, 128], f32, name="io_t")
        nc.sync.dma_start(io_t[:], iota)
        on_t = sb.tile([128, 1], f32, name="on_t")
        nc.sync.dma_start(on_t[:], ones)
        rb_t = sb.tile([PL, 1], f32, name="rb_t")
        nc.sync.dma_start(rb_t[:], rbase)

        # labels replicated over s: plane p=(b,s,k) gets labels[b,k,:]
        lblr = sb.tile([PL, 11], f32, name="lblr")
        for b in range(BL):
            for s in range(S):
                p0 = (b * S + s) * K
                nc.sync.dma_start(lblr[p0 : p0 + K, :], lbl[b])
        # the 9 used label-pred elements: lb[b,s,k,0,0:9]
        pred9 = sb.tile([PL, 9], f32, name="pred9")
        for b in range(BL):
            for s in range(S):
                p0 = (b * S + s) * K
                nc.sync.dma_start(pred9[p0 : p0 + K, :], cp[b, s, K:C2, 0, 0:9])

        # ---- big loads: ht per b, hm per group; h on partitions ----
        acc_ht = sb.tile([128, BL], f32, name="acc_ht")
        ht_ts = []
        for b in range(BL):
            ht_t = sb.tile([128, KW], f32, name=f"ht{b}")
            nc.sync.dma_start(
                ht_t[:].rearrange("h (k w) -> h k w", k=K),
                hmr[b].rearrange("k h w -> h k w"),
            )
            ht_ts.append(ht_t)
        for b in range(BL):
            sqh = scr.tile([128, KW], f32, name=f"sqh{b}", tag="sq")
            nc.scalar.activation(
                out=sqh[:], in_=ht_ts[b][:], func=Act.Square,
                accum_out=acc_ht[:, b : b + 1],
            )

        R_all = sb.tile([128, PL], f32, name="R_all")
        acc_hm = sb.tile([128, G], f32, name="acc_hm")
        diag = sb.tile([128, G], f32, name="diag")

        hm_ts = []
        for g in range(G):
            b, s = divmod(g, S)
            hm_t = sb.tile([128, KW], f32, name=f"hm{g}")
            nc.sync.dma_start(
                hm_t[:].rearrange("h (k w) -> h k w", k=K),
                cp[b, s, 0:K].rearrange("k h w -> h k w"),
            )
            hm_ts.append(hm_t)
            # per-(plane,h) max over w -> columns of R_all
            nc.vector.tensor_reduce(
                out=R_all[:, g * K : (g + 1) * K],
                in_=hm_t[:].rearrange("h (k w) -> h k w", k=K),
                axis=Ax.X,
                op=Alu.max,
            )
            # sum_f hm^2 per partition
            sq = scr.tile([128, KW], f32, name=f"sq{g}", tag="sq")
            nc.scalar.activation(
                out=sq[:], in_=hm_t[:], func=Act.Square,
                accum_out=acc_hm[:, g : g + 1],
            )

        # ---- cross terms on PE: for each (b,s), accumulate over k the
        # matmul ht_k^T @ hm_k; only the PSUM diagonal is needed ----
        for b in range(BL):
            psx = {}
            for s in range(S):
                psx[s] = ps.tile([128, 128], f32, name=f"px{b}{s}", tag=f"px{s}")
            for k in range(K):
                sl = slice(k * 128, (k + 1) * 128)
                for s in range(S):
                    nc.tensor.matmul(
                        out=psx[s][:],
                        lhsT=ht_ts[b][:, sl],
                        rhs=hm_ts[b * S + s][:, sl],
                        start=(k == 0),
                        stop=(k == K - 1),
                        skip_group_check=True,
                    )
            for s in range(S):
                g = b * S + s
                dsc = scr.tile([128, 128], f32, name=f"dsc{g}", tag="dsc")
                nc.vector.scalar_tensor_tensor(
                    out=dsc[:],
                    in0=psx[s][:],
                    scalar=1.0,
                    in1=id_t[:],
                    op0=Alu.bypass,
                    op1=Alu.mult,
                    accum_out=diag[:, g : g + 1],
                )

        # ---- combine per-partition heat pieces, then partition-sum ----
        ucomb = sb.tile([128, G], f32, name="ucomb")
        for b in range(BL):
            nc.vector.tensor_tensor(
                out=ucomb[:, b * S : (b + 1) * S],
                in0=acc_hm[:, b * S : (b + 1) * S],
                in1=acc_ht[:, b : b + 1].to_broadcast([128, S]),
                op=Alu.add,
            )
        acc_fin = sb.tile([128, G], f32, name="acc_fin")
        nc.vector.scalar_tensor_tensor(
            out=acc_fin[:], in0=diag[:], scalar=-2.0, in1=ucomb[:],
            op0=Alu.mult, op1=Alu.add,
        )
        psum_hs = ps.tile([1, G], f32, name="psum_hs", tag="hs")
        nc.tensor.matmul(
            out=psum_hs[:], lhsT=on_t[:], rhs=acc_fin[:], start=True, stop=True
        )
        heat_row = sb.tile([1, G], f32, name="heat_row")
        nc.vector.tensor_copy(out=heat_row[:], in_=psum_hs[:])
        nc.sync.dma_start(out_heat, heat_row[:])

        # ---- global max + argmax per plane ----
        psum_rt = ps.tile([PL, 128], f32, name="psum_rt", tag="rt")
        nc.tensor.transpose(out=psum_rt[:], in_=R_all[:], identity=id_t[:])
        Mv = sb.tile([PL, 1], f32, name="Mv")
        nc.vector.tensor_reduce(out=Mv[:], in_=psum_rt[:], axis=Ax.X, op=Alu.max)
        xsc = scr.tile([PL, 128], f32, name="xsc", tag="xysc")
        xf = sb.tile([PL, 1], f32, name="xf")
        nc.vector.scalar_tensor_tensor(
            out=xsc[:], in0=psum_rt[:], scalar=Mv[:, 0:1], in1=io_t[0:PL, :],
            op0=Alu.is_equal, op1=Alu.mult, accum_out=xf[:],
        )
        # gather row x of each plane straight from DRAM
        ridf = sb.tile([PL, 1], f32, name="ridf")
        nc.vector.tensor_tensor(out=ridf[:], in0=xf[:], in1=rb_t[:], op=Alu.add)
        ridu = sb.tile([PL, 1], mybir.dt.uint32, name="ridu")
        nc.vector.tensor_copy(out=ridu[:], in_=ridf[:])
        gath = sb.tile([PL, 128], f32, name="gath")
        nc.gpsimd.indirect_dma_start(
            out=gath[:],
            out_offset=None,
            in_=cp.rearrange("b s c h w -> (b s c h) w"),
            in_offset=bass.IndirectOffsetOnAxis(ap=ridu[:, 0:1], axis=0),
        )
        ysc = scr.tile([PL, 128], f32, name="ysc", tag="xysc")
        yf = sb.tile([PL, 1], f32, name="yf")
        nc.vector.scalar_tensor_tensor(
            out=ysc[:], in0=gath[:], scalar=Mv[:, 0:1], in1=io_t[0:PL, :],
            op0=Alu.is_equal, op1=Alu.mult, accum_out=yf[:],
        )

        # ---- per-plane label loss ----
        cdiff = sb.tile([PL, 7], f32, name="cdiff")
        nc.vector.tensor_tensor(
            out=cdiff[:], in0=pred9[:, 0:7], in1=lblr[:, 0:7], op=Alu.subtract
        )
        csc = sb.tile([PL, 7], f32, name="csc")
        cls = sb.tile([PL, 1], f32, name="cls")
        nc.scalar.activation(
            out=csc[:], in_=cdiff[:], func=Act.Square, accum_out=cls[:]
        )
        conf = sb.tile([PL, 1], f32, name="conf")
        nc.scalar.activation(
            out=conf[:], in_=Mv[:], func=Act.Square, bias=1.0, scale=-1.0
        )
        # xy loss
        t1 = sb.tile([PL, 1], f32, name="t1")
        nc.vector.tensor_tensor(t1[:], lblr[:, 9:10], lblr[:, 7:8], Alu.add)
        t2 = sb.tile([PL, 1], f32, name="t2")
        nc.vector.tensor_tensor(t2[:], xf[:], pred9[:, 7:8], Alu.add)
        tx = sb.tile([PL, 1], f32, name="tx")
        nc.vector.tensor_tensor(tx[:], t1[:], t2[:], Alu.subtract)
        t3 = sb.tile([PL, 1], f32, name="t3")
        nc.vector.tensor_tensor(t3[:], lblr[:, 10:11], lblr[:, 8:9], Alu.add)
        t4 = sb.tile([PL, 1], f32, name="t4")
        nc.vector.tensor_tensor(t4[:], yf[:], pred9[:, 8:9], Alu.add)
        ty = sb.tile([PL, 1], f32, name="ty")
        nc.vector.tensor_tensor(ty[:], t3[:], t4[:], Alu.subtract)
        txs = sb.tile([PL, 1], f32, name="txs")
        nc.vector.tensor_tensor(txs[:], tx[:], tx[:], Alu.mult)
        xyl = sb.tile([PL, 1], f32, name="xyl")
        nc.vector.scalar_tensor_tensor(
            out=xyl[:], in0=ty[:], scalar=ty[:, 0:1], in1=txs[:],
            op0=Alu.mult, op1=Alu.add,
        )
        # validity mask: (gx>0)&(gy>0)&(gx<128)&(gy<128)
        gmin = sb.tile([PL, 1], f32, name="gmin")
        nc.vector.tensor_tensor(gmin[:], lblr[:, 9:10], lblr[:, 10:11], Alu.min)
        gmax = sb.tile([PL, 1], f32, name="gmax")
        nc.vector.tensor_tensor(gmax[:], lblr[:, 9:10], lblr[:, 10:11], Alu.max)
        c1 = sb.tile([PL, 1], f32, name="c1")
        nc.vector.tensor_scalar(c1[:], gmin[:], 0.0, None, Alu.is_gt)
        c2t = sb.tile([PL, 1], f32, name="c2t")
        nc.vector.tensor_scalar(c2t[:], gmax[:], float(H), None, Alu.is_lt)
        vv = sb.tile([PL, 1], f32, name="vv")
        nc.vector.tensor_tensor(vv[:], c1[:], c2t[:], Alu.mult)
        # total, masked
        tot = sb.tile([PL, 1], f32, name="tot")
        nc.vector.tensor_tensor(tot[:], cls[:], xyl[:], Alu.add)
        tot2 = sb.tile([PL, 1], f32, name="tot2")
        nc.vector.tensor_tensor(tot2[:], tot[:], conf[:], Alu.add)
        perkp = sb.tile([PL, 1], f32, name="perkp")
        nc.vector.tensor_tensor(perkp[:], tot2[:], vv[:], Alu.mult)
        # reduce over k: transpose to one row, then segment-sum
        psum_lk = ps.tile([1, PL], f32, name="psum_lk", tag="lk")
        nc.tensor.transpose(
            out=psum_lk[:], in_=perkp[:], identity=id_t[0:PL, 0:PL]
        )
        lab_row = sb.tile([1, G], f32, name="lab_row")
        nc.vector.tensor_reduce(
            out=lab_row[:],
            in_=psum_lk[:].rearrange("o (g k) -> o g k", k=K),
            axis=Ax.X,
            op=Alu.add,
        )
        nc.sync.dma_start(out_label, lab_row[:])

    nc.compile()
    return nc


def _get_nc():
    if "nc" not in _CACHE:
        _CACHE["nc"] = _build_module()
    return _CACHE["nc"]


def _in_maps(combined_preds, heatmaps, labels):
    cp = np.ascontiguousarray(combined_preds, dtype=np.float32)
    hmr = np.ascontiguousarray(heatmaps, dtype=np.float32)
    lb = np.ascontiguousarray(labels, dtype=np.float32)
    maps = []
    for i in range(NCORES):
        b0 = BL * i
        maps.append(
            {
                "cp": np.ascontiguousarray(cp[b0 : b0 + BL]),
                "hmr": np.ascontiguousarray(hmr[b0 : b0 + BL]),
                "lbl": np.ascontiguousarray(lb[b0 : b0 + BL]),
            }
        )
    return maps


def run(combined_preds, heatmaps, labels, trace=False):
    """Run on hardware; returns ((heat, label), BassKernelResults)."""
    from concourse import bass_utils

    nc = _get_nc()
    res = bass_utils.run_bass_kernel_spmd(
        nc,
        _in_maps(combined_preds, heatmaps, labels),
        core_ids=list(range(NCORES)),
        trace=trace,
    )
    heat = np.concatenate(
        [res.results[i]["out_heat"].reshape(BL, S) for i in range(NCORES)], axis=0
    )
    lab = np.concatenate(
        [res.results[i]["out_label"].reshape(BL, S) for i in range(NCORES)], axis=0
    )
    return (heat, lab), res


def kernel(combined_preds, heatmaps, labels):
    (heat, lab), _ = run(combined_preds, heatmaps, labels)
    return heat, lab
